# revision 3
# baseline (speedup 1.0000x reference)
"""Trilinear 3D-LUT apply (Generator3DLUT identity) on 8 Trainium2 cores.

Strategy
--------
The reference op is trilinear interpolation of a 3x33x33x33 LUT at the
pixel RGB coordinates.  When the LUT is affine in its (b,g,r) indices --
which we verify exactly at runtime on the host -- trilinear interpolation
collapses algebraically to a per-pixel affine map:

    out[c] = bias_c + sum_k M[c,k] * (x[k] / binsize)

For a *diagonal* M (e.g. the identity LUT) this is a single fused
multiply-add per element, which we run data-parallel over the batch on
8 NeuronCores as a Bass/Tile kernel (DMA-bound streaming kernel).

If the LUT is not affine (never the case for this problem's inputs) we
fall back to an exact numpy gather implementation on the host.

Dtype: the on-device stream runs in bf16 by default (rel. rounding error
~4e-3, far inside the 2e-2 gate) which halves HBM traffic; set
LUT_KERNEL_DTYPE=f32 for the full-precision stream.
"""

import os

import numpy as np

DIM = 33
_B, _C, _H, _W = 8, 3, 1024, 1024
_N_CORES = 8
_BINSIZE = np.float32(1.0001) / np.float32(DIM - 1)

# ---------------------------------------------------------------------------
# Host-side LUT analysis
# ---------------------------------------------------------------------------


def _affine_from_lut(LUT):
    """If trilinear interp of LUT is exactly affine, return (M, bias).

    LUT[c, b, g, r] affine in indices means
      LUT[c] = bias_c + M[c,0]*r + M[c,1]*g + M[c,2]*b
    and then trilinear interpolation at scaled coords (r_s, g_s, b_s)
    evaluates to bias_c + M[c,0]*r_s + M[c,1]*g_s + M[c,2]*b_s.
    Returns None if the LUT is not affine to fp32 exactness.
    """
    dim = LUT.shape[-1]
    idx = np.arange(dim, dtype=np.float32)
    M = np.empty((3, 3), np.float64)
    bias = np.empty(3, np.float64)
    for c in range(3):
        L = LUT[c]
        a = np.float64(L[0, 0, 0])
        br = np.float64(L[0, 0, 1]) - a
        bg = np.float64(L[0, 1, 0]) - a
        bb = np.float64(L[1, 0, 0]) - a
        pred = (
            a
            + br * idx[None, None, :]
            + bg * idx[None, :, None]
            + bb * idx[:, None, None]
        ).astype(np.float32)
        if not np.allclose(pred, L, rtol=0, atol=1e-6):
            return None
        M[c] = (br, bg, bb)
        bias[c] = a
    return M, bias


def _trilinear_np(LUT, x):
    """Exact numpy fallback (general LUT)."""
    dim = LUT.shape[-1]
    inv = np.float32(1.0) / _BINSIZE
    lut_flat = np.ascontiguousarray(LUT.reshape(3, dim * dim * dim))
    out = np.empty_like(x)
    for i in range(x.shape[0]):
        r, g, b = x[i, 0], x[i, 1], x[i, 2]
        r_s, g_s, b_s = r * inv, g * inv, b * inv
        r_id = np.clip(np.floor(r_s), 0, dim - 2).astype(np.int32)
        g_id = np.clip(np.floor(g_s), 0, dim - 2).astype(np.int32)
        b_id = np.clip(np.floor(b_s), 0, dim - 2).astype(np.int32)
        r_d = r_s - r_id.astype(np.float32)
        g_d = g_s - g_id.astype(np.float32)
        b_d = b_s - b_id.astype(np.float32)
        base = r_id + g_id * dim + b_id * (dim * dim)
        acc = np.zeros((3,) + r.shape, np.float32)
        for db in (0, 1):
            wb = b_d if db else 1.0 - b_d
            for dg in (0, 1):
                wg = g_d if dg else 1.0 - g_d
                for dr in (0, 1):
                    wr = r_d if dr else 1.0 - r_d
                    flat = base + (dr + dg * dim + db * dim * dim)
                    v = lut_flat[:, flat.ravel()].reshape((3,) + r.shape)
                    acc += (wr * wg * wb)[None].astype(np.float32) * v
        out[i] = acc
    return out


# ---------------------------------------------------------------------------
# Bass kernel: per-core streaming out[c] = s_c * x[c] + b_c
# ---------------------------------------------------------------------------

_compiled = {}


def _build_scale_kernel(scales, biases, dtype_tag, tile_f, bufs):
    """Per-core program: x [3,1024,1024] -> out = s_c*x[c]+b_c, streamed."""
    import concourse.bacc as bacc
    import concourse.mybir as mybir
    from concourse.tile import TileContext

    dt = mybir.dt.bfloat16 if dtype_tag == "bf16" else mybir.dt.float32

    nc = bacc.Bacc(
        "TRN2",
        target_bir_lowering=False,
        debug=False,
        num_devices=_N_CORES,
    )
    x = nc.dram_tensor("x", [_C, _H, _W], dt, kind="ExternalInput").ap()
    out = nc.dram_tensor("out", [_C, _H, _W], dt, kind="ExternalOutput").ap()

    free_per_chan = _H * _W // 128  # 8192
    assert free_per_chan % tile_f == 0
    ntiles = free_per_chan // tile_f

    with TileContext(nc) as tc:
        with tc.tile_pool(name="io", bufs=bufs) as pool:
            for c in range(_C):
                xin = x[c].rearrange("(p a) w -> p (a w)", p=128)
                xout = out[c].rearrange("(p a) w -> p (a w)", p=128)
                for j in range(ntiles):
                    sl = slice(j * tile_f, (j + 1) * tile_f)
                    t = pool.tile([128, tile_f], dt)
                    nc.sync.dma_start(t[:], xin[:, sl])
                    nc.vector.tensor_scalar(
                        t[:],
                        t[:],
                        float(scales[c]),
                        float(biases[c]),
                        mybir.AluOpType.mult,
                        mybir.AluOpType.add,
                    )
                    nc.sync.dma_start(xout[:, sl], t[:])
    nc.compile()
    return nc


def _get_scale_kernel(scales, biases, dtype_tag, tile_f=2048, bufs=6):
    key = (
        dtype_tag,
        tile_f,
        bufs,
        tuple(np.float32(s) for s in scales),
        tuple(np.float32(b) for b in biases),
    )
    if key not in _compiled:
        _compiled[key] = _build_scale_kernel(scales, biases, dtype_tag, tile_f, bufs)
    return _compiled[key]


LAST_RESULTS = None  # BassKernelResults of the most recent device run
PHASE_NS = None  # phase wall timings of the most recent device run


def _run_diag_affine(x, scales, biases, dtype_tag):
    global LAST_RESULTS, PHASE_NS
    import time

    from concourse.bass_utils import run_bass_kernel_spmd

    t0 = time.time()
    nc = _get_scale_kernel(scales, biases, dtype_tag)
    t1 = time.time()

    if dtype_tag == "bf16":
        import ml_dtypes

        xs = x.astype(ml_dtypes.bfloat16)
    else:
        xs = x
    in_maps = [{"x": xs[i]} for i in range(_B)]
    t2 = time.time()
    res = run_bass_kernel_spmd(nc, in_maps, core_ids=list(range(_N_CORES)))
    t3 = time.time()
    LAST_RESULTS = res
    out = np.stack([res.results[i]["out"] for i in range(_B)])
    out = out.astype(np.float32, copy=False)
    t4 = time.time()
    PHASE_NS = {
        "build": int((t1 - t0) * 1e9),
        "convert": int((t2 - t1) * 1e9),
        "spmd": int((t3 - t2) * 1e9),
        "post": int((t4 - t3) * 1e9),
    }
    return out


# ---------------------------------------------------------------------------
# Entry point
# ---------------------------------------------------------------------------


def kernel(LUT=None, x=None, **kwargs):
    LUT = np.asarray(LUT, dtype=np.float32)
    x = np.ascontiguousarray(np.asarray(x, dtype=np.float32))
    assert x.shape == (_B, _C, _H, _W), x.shape

    aff = _affine_from_lut(LUT)
    if aff is not None:
        M, bias = aff
        offdiag = M - np.diag(np.diag(M))
        if np.all(offdiag == 0.0):
            scales = np.diag(M) / np.float64(_BINSIZE)
            try:
                return _run_diag_affine(
                    x, scales, bias, os.environ.get("LUT_KERNEL_DTYPE", "bf16")
                )
            except Exception:
                import traceback

                traceback.print_exc()
        # general affine (or device failure): exact on host
        inv = 1.0 / np.float64(_BINSIZE)
        xs = x * np.float32(inv)
        out = np.einsum("ck,bkhw->bchw", M.astype(np.float32), xs)
        out += bias.astype(np.float32)[None, :, None, None]
        return out.astype(np.float32, copy=False)

    return _trilinear_np(LUT, x)


# revision 8
# speedup vs baseline: 1.2231x; 1.2231x over previous
"""Trilinear 3D-LUT apply (Generator3DLUT identity) on 8 Trainium2 cores.

Strategy
--------
The reference op is trilinear interpolation of a 3x33x33x33 LUT at the
pixel RGB coordinates.  When the LUT is affine in its (b,g,r) indices --
which we verify exactly at runtime on the host -- trilinear interpolation
collapses algebraically to a per-pixel affine map:

    out[c] = bias_c + sum_k M[c,k] * (x[k] / binsize)

For a *diagonal* M (e.g. the identity LUT) this is a single fused
multiply-add per element, which we run data-parallel over the batch on
8 NeuronCores as a Bass/Tile kernel (DMA-bound streaming kernel).

If the LUT is not affine (never the case for this problem's inputs) we
fall back to an exact numpy gather implementation on the host.

Dtype: the on-device stream runs in bf16 by default (rel. rounding error
~4e-3, far inside the 2e-2 gate) which halves HBM traffic; set
LUT_KERNEL_DTYPE=f32 for the full-precision stream.
"""

import os

import numpy as np

DIM = 33
_B, _C, _H, _W = 8, 3, 1024, 1024
_N_CORES = 8
_BINSIZE = np.float32(1.0001) / np.float32(DIM - 1)

# ---------------------------------------------------------------------------
# Host-side LUT analysis
# ---------------------------------------------------------------------------


def _affine_from_lut(LUT):
    """If trilinear interp of LUT is exactly affine, return (M, bias).

    LUT[c, b, g, r] affine in indices means
      LUT[c] = bias_c + M[c,0]*r + M[c,1]*g + M[c,2]*b
    and then trilinear interpolation at scaled coords (r_s, g_s, b_s)
    evaluates to bias_c + M[c,0]*r_s + M[c,1]*g_s + M[c,2]*b_s.
    Returns None if the LUT is not affine to fp32 exactness.
    """
    dim = LUT.shape[-1]
    idx = np.arange(dim, dtype=np.float32)
    M = np.empty((3, 3), np.float64)
    bias = np.empty(3, np.float64)
    for c in range(3):
        L = LUT[c]
        a = np.float64(L[0, 0, 0])
        br = np.float64(L[0, 0, 1]) - a
        bg = np.float64(L[0, 1, 0]) - a
        bb = np.float64(L[1, 0, 0]) - a
        pred = (
            a
            + br * idx[None, None, :]
            + bg * idx[None, :, None]
            + bb * idx[:, None, None]
        ).astype(np.float32)
        if not np.allclose(pred, L, rtol=0, atol=1e-6):
            return None
        M[c] = (br, bg, bb)
        bias[c] = a
    return M, bias


def _trilinear_np(LUT, x):
    """Exact numpy fallback (general LUT)."""
    dim = LUT.shape[-1]
    inv = np.float32(1.0) / _BINSIZE
    lut_flat = np.ascontiguousarray(LUT.reshape(3, dim * dim * dim))
    out = np.empty_like(x)
    for i in range(x.shape[0]):
        r, g, b = x[i, 0], x[i, 1], x[i, 2]
        r_s, g_s, b_s = r * inv, g * inv, b * inv
        r_id = np.clip(np.floor(r_s), 0, dim - 2).astype(np.int32)
        g_id = np.clip(np.floor(g_s), 0, dim - 2).astype(np.int32)
        b_id = np.clip(np.floor(b_s), 0, dim - 2).astype(np.int32)
        r_d = r_s - r_id.astype(np.float32)
        g_d = g_s - g_id.astype(np.float32)
        b_d = b_s - b_id.astype(np.float32)
        base = r_id + g_id * dim + b_id * (dim * dim)
        acc = np.zeros((3,) + r.shape, np.float32)
        for db in (0, 1):
            wb = b_d if db else 1.0 - b_d
            for dg in (0, 1):
                wg = g_d if dg else 1.0 - g_d
                for dr in (0, 1):
                    wr = r_d if dr else 1.0 - r_d
                    flat = base + (dr + dg * dim + db * dim * dim)
                    v = lut_flat[:, flat.ravel()].reshape((3,) + r.shape)
                    acc += (wr * wg * wb)[None].astype(np.float32) * v
        out[i] = acc
    return out


# ---------------------------------------------------------------------------
# Bass kernel: per-core streaming out[c] = s_c * x[c] + b_c
# ---------------------------------------------------------------------------

_compiled = {}


def _build_scale_kernel(
    scales, biases, dtype_tag, tile_f, bufs, engines="v", flat=False, store_act=False
):
    """Per-core program: x [3,1024,1024] -> out = s_c*x[c]+b_c, streamed.

    engines: string over {'v','s','g'} -- tiles round-robin across the
    named compute engines (vector / scalar-ACT / gpsimd).
    flat: treat the whole image as one uniform stream (requires all
    scales equal and all biases equal).
    """
    import concourse.bacc as bacc
    import concourse.mybir as mybir
    from concourse.tile import TileContext

    dt = mybir.dt.bfloat16 if dtype_tag == "bf16" else mybir.dt.float32

    nc = bacc.Bacc(
        "TRN2",
        target_bir_lowering=False,
        debug=False,
        num_devices=_N_CORES,
    )
    x = nc.dram_tensor("x", [_C, _H, _W], dt, kind="ExternalInput").ap()
    out = nc.dram_tensor("out", [_C, _H, _W], dt, kind="ExternalOutput").ap()

    def compute(engine_code, t, s, b):
        if engine_code == "v":
            nc.vector.tensor_scalar(
                t[:], t[:], s, b, mybir.AluOpType.mult, mybir.AluOpType.add
            )
        elif engine_code == "g":
            nc.gpsimd.tensor_scalar(
                t[:], t[:], s, b, mybir.AluOpType.mult, mybir.AluOpType.add
            )
        elif engine_code == "s":
            if b == 0.0:
                nc.scalar.mul(t[:], t[:], s)
            else:
                nc.scalar.activation(
                    t[:],
                    t[:],
                    mybir.ActivationFunctionType.Copy,
                    bias=b,
                    scale=s,
                )
        else:
            raise ValueError(engine_code)

    store_eng = nc.scalar if store_act else nc.sync
    k = 0
    with TileContext(nc) as tc:
        with tc.tile_pool(name="io", bufs=bufs) as pool:
            if flat:
                assert len(set(map(float, scales))) == 1
                assert len(set(map(float, biases))) == 1
                free_total = _C * _H * _W // 128  # 24576
                assert free_total % tile_f == 0
                xin = x.rearrange("c (p a) w -> p (c a w)", p=128)
                xout = out.rearrange("c (p a) w -> p (c a w)", p=128)
                for j in range(free_total // tile_f):
                    sl = slice(j * tile_f, (j + 1) * tile_f)
                    t = pool.tile([128, tile_f], dt)
                    nc.sync.dma_start(t[:], xin[:, sl])
                    compute(engines[k % len(engines)], t, float(scales[0]), float(biases[0]))
                    k += 1
                    store_eng.dma_start(xout[:, sl], t[:])
            else:
                free_per_chan = _H * _W // 128  # 8192
                assert free_per_chan % tile_f == 0
                ntiles = free_per_chan // tile_f
                for c in range(_C):
                    xin = x[c].rearrange("(p a) w -> p (a w)", p=128)
                    xout = out[c].rearrange("(p a) w -> p (a w)", p=128)
                    for j in range(ntiles):
                        sl = slice(j * tile_f, (j + 1) * tile_f)
                        t = pool.tile([128, tile_f], dt)
                        nc.sync.dma_start(t[:], xin[:, sl])
                        compute(
                            engines[k % len(engines)],
                            t,
                            float(scales[c]),
                            float(biases[c]),
                        )
                        k += 1
                        store_eng.dma_start(xout[:, sl], t[:])
    nc.compile()
    return nc


def _get_scale_kernel(
    scales, biases, dtype_tag, tile_f=2048, bufs=12, engines="v", store_act=True
):
    key = (
        dtype_tag,
        tile_f,
        bufs,
        engines,
        store_act,
        tuple(np.float32(s) for s in scales),
        tuple(np.float32(b) for b in biases),
    )
    if key not in _compiled:
        _compiled[key] = _build_scale_kernel(
            scales, biases, dtype_tag, tile_f, bufs, engines, False, store_act
        )
    return _compiled[key]


LAST_RESULTS = None  # BassKernelResults of the most recent device run
PHASE_NS = None  # phase wall timings of the most recent device run


def _run_diag_affine(x, scales, biases, dtype_tag):
    global LAST_RESULTS, PHASE_NS
    import time

    from concourse.bass_utils import run_bass_kernel_spmd

    t0 = time.time()
    nc = _get_scale_kernel(scales, biases, dtype_tag)
    t1 = time.time()

    if dtype_tag == "bf16":
        import ml_dtypes

        xs = x.astype(ml_dtypes.bfloat16)
    else:
        xs = x
    in_maps = [{"x": xs[i]} for i in range(_B)]
    t2 = time.time()
    res = run_bass_kernel_spmd(nc, in_maps, core_ids=list(range(_N_CORES)))
    t3 = time.time()
    LAST_RESULTS = res
    out = np.empty((_B, _C, _H, _W), np.float32)
    for i in range(_B):
        out[i] = res.results[i]["out"]  # casts bf16 -> f32 in one pass
    t4 = time.time()
    PHASE_NS = {
        "build": int((t1 - t0) * 1e9),
        "convert": int((t2 - t1) * 1e9),
        "spmd": int((t3 - t2) * 1e9),
        "post": int((t4 - t3) * 1e9),
    }
    return out


# ---------------------------------------------------------------------------
# Entry point
# ---------------------------------------------------------------------------


def kernel(LUT=None, x=None, **kwargs):
    LUT = np.asarray(LUT, dtype=np.float32)
    x = np.ascontiguousarray(np.asarray(x, dtype=np.float32))
    assert x.shape == (_B, _C, _H, _W), x.shape

    aff = _affine_from_lut(LUT)
    if aff is not None:
        M, bias = aff
        offdiag = M - np.diag(np.diag(M))
        if np.all(offdiag == 0.0):
            scales = np.diag(M) / np.float64(_BINSIZE)
            try:
                return _run_diag_affine(
                    x, scales, bias, os.environ.get("LUT_KERNEL_DTYPE", "bf16")
                )
            except Exception:
                import traceback

                traceback.print_exc()
        # general affine (or device failure): exact on host
        inv = 1.0 / np.float64(_BINSIZE)
        xs = x * np.float32(inv)
        out = np.einsum("ck,bkhw->bchw", M.astype(np.float32), xs)
        out += bias.astype(np.float32)[None, :, None, None]
        return out.astype(np.float32, copy=False)

    return _trilinear_np(LUT, x)


# revision 9
# speedup vs baseline: 1.2646x; 1.0339x over previous
"""Trilinear 3D-LUT apply (Generator3DLUT identity) on 8 Trainium2 cores.

Strategy
--------
The reference op is trilinear interpolation of a 3x33x33x33 LUT at the
pixel RGB coordinates.  When the LUT is affine in its (b,g,r) indices --
which we verify exactly at runtime on the host -- trilinear interpolation
collapses algebraically to a per-pixel affine map:

    out[c] = bias_c + sum_k M[c,k] * (x[k] / binsize)

For a *diagonal* M (e.g. the identity LUT) this is a single fused
multiply-add per element, which we run data-parallel over the batch on
8 NeuronCores as a Bass/Tile kernel (DMA-bound streaming kernel).

If the LUT is not affine (never the case for this problem's inputs) we
fall back to an exact numpy gather implementation on the host.

Dtype: the on-device stream runs in bf16 by default (rel. rounding error
~4e-3, far inside the 2e-2 gate) which halves HBM traffic; set
LUT_KERNEL_DTYPE=f32 for the full-precision stream.
"""

import os

import numpy as np

DIM = 33
_B, _C, _H, _W = 8, 3, 1024, 1024
_N_CORES = 8
_BINSIZE = np.float32(1.0001) / np.float32(DIM - 1)

# ---------------------------------------------------------------------------
# Host-side LUT analysis
# ---------------------------------------------------------------------------


def _affine_from_lut(LUT):
    """If trilinear interp of LUT is exactly affine, return (M, bias).

    LUT[c, b, g, r] affine in indices means
      LUT[c] = bias_c + M[c,0]*r + M[c,1]*g + M[c,2]*b
    and then trilinear interpolation at scaled coords (r_s, g_s, b_s)
    evaluates to bias_c + M[c,0]*r_s + M[c,1]*g_s + M[c,2]*b_s.
    Returns None if the LUT is not affine to fp32 exactness.
    """
    dim = LUT.shape[-1]
    idx = np.arange(dim, dtype=np.float32)
    M = np.empty((3, 3), np.float64)
    bias = np.empty(3, np.float64)
    for c in range(3):
        L = LUT[c]
        a = np.float64(L[0, 0, 0])
        br = np.float64(L[0, 0, 1]) - a
        bg = np.float64(L[0, 1, 0]) - a
        bb = np.float64(L[1, 0, 0]) - a
        pred = (
            a
            + br * idx[None, None, :]
            + bg * idx[None, :, None]
            + bb * idx[:, None, None]
        ).astype(np.float32)
        if not np.allclose(pred, L, rtol=0, atol=1e-6):
            return None
        M[c] = (br, bg, bb)
        bias[c] = a
    return M, bias


def _trilinear_np(LUT, x):
    """Exact numpy fallback (general LUT)."""
    dim = LUT.shape[-1]
    inv = np.float32(1.0) / _BINSIZE
    lut_flat = np.ascontiguousarray(LUT.reshape(3, dim * dim * dim))
    out = np.empty_like(x)
    for i in range(x.shape[0]):
        r, g, b = x[i, 0], x[i, 1], x[i, 2]
        r_s, g_s, b_s = r * inv, g * inv, b * inv
        r_id = np.clip(np.floor(r_s), 0, dim - 2).astype(np.int32)
        g_id = np.clip(np.floor(g_s), 0, dim - 2).astype(np.int32)
        b_id = np.clip(np.floor(b_s), 0, dim - 2).astype(np.int32)
        r_d = r_s - r_id.astype(np.float32)
        g_d = g_s - g_id.astype(np.float32)
        b_d = b_s - b_id.astype(np.float32)
        base = r_id + g_id * dim + b_id * (dim * dim)
        acc = np.zeros((3,) + r.shape, np.float32)
        for db in (0, 1):
            wb = b_d if db else 1.0 - b_d
            for dg in (0, 1):
                wg = g_d if dg else 1.0 - g_d
                for dr in (0, 1):
                    wr = r_d if dr else 1.0 - r_d
                    flat = base + (dr + dg * dim + db * dim * dim)
                    v = lut_flat[:, flat.ravel()].reshape((3,) + r.shape)
                    acc += (wr * wg * wb)[None].astype(np.float32) * v
        out[i] = acc
    return out


# ---------------------------------------------------------------------------
# Bass kernel: per-core streaming out[c] = s_c * x[c] + b_c
# ---------------------------------------------------------------------------

_compiled = {}


def _build_scale_kernel(
    scales, biases, dtype_tag, tile_f, bufs, engines="v", flat=False, store_act=False
):
    """Per-core program: x [3,1024,1024] -> out = s_c*x[c]+b_c, streamed.

    engines: string over {'v','s','g'} -- tiles round-robin across the
    named compute engines (vector / scalar-ACT / gpsimd).
    flat: treat the whole image as one uniform stream (requires all
    scales equal and all biases equal).
    """
    import concourse.bacc as bacc
    import concourse.mybir as mybir
    from concourse.tile import TileContext

    dt = mybir.dt.bfloat16 if dtype_tag == "bf16" else mybir.dt.float32

    nc = bacc.Bacc(
        "TRN2",
        target_bir_lowering=False,
        debug=False,
        num_devices=_N_CORES,
    )
    x = nc.dram_tensor("x", [_C, _H, _W], dt, kind="ExternalInput").ap()
    out = nc.dram_tensor("out", [_C, _H, _W], dt, kind="ExternalOutput").ap()

    def compute(engine_code, t, s, b):
        if engine_code == "v":
            nc.vector.tensor_scalar(
                t[:], t[:], s, b, mybir.AluOpType.mult, mybir.AluOpType.add
            )
        elif engine_code == "g":
            nc.gpsimd.tensor_scalar(
                t[:], t[:], s, b, mybir.AluOpType.mult, mybir.AluOpType.add
            )
        elif engine_code == "s":
            if b == 0.0:
                nc.scalar.mul(t[:], t[:], s)
            else:
                nc.scalar.activation(
                    t[:],
                    t[:],
                    mybir.ActivationFunctionType.Copy,
                    bias=b,
                    scale=s,
                )
        else:
            raise ValueError(engine_code)

    store_eng = nc.scalar if store_act else nc.sync
    k = 0
    with TileContext(nc) as tc:
        with tc.tile_pool(name="io", bufs=bufs) as pool:
            if flat:
                assert len(set(map(float, scales))) == 1
                assert len(set(map(float, biases))) == 1
                free_total = _C * _H * _W // 128  # 24576
                assert free_total % tile_f == 0
                xin = x.rearrange("c (p a) w -> p (c a w)", p=128)
                xout = out.rearrange("c (p a) w -> p (c a w)", p=128)
                for j in range(free_total // tile_f):
                    sl = slice(j * tile_f, (j + 1) * tile_f)
                    t = pool.tile([128, tile_f], dt)
                    nc.sync.dma_start(t[:], xin[:, sl])
                    compute(engines[k % len(engines)], t, float(scales[0]), float(biases[0]))
                    k += 1
                    store_eng.dma_start(xout[:, sl], t[:])
            else:
                free_per_chan = _H * _W // 128  # 8192
                assert free_per_chan % tile_f == 0
                ntiles = free_per_chan // tile_f
                for c in range(_C):
                    xin = x[c].rearrange("(p a) w -> p (a w)", p=128)
                    xout = out[c].rearrange("(p a) w -> p (a w)", p=128)
                    for j in range(ntiles):
                        sl = slice(j * tile_f, (j + 1) * tile_f)
                        t = pool.tile([128, tile_f], dt)
                        nc.sync.dma_start(t[:], xin[:, sl])
                        compute(
                            engines[k % len(engines)],
                            t,
                            float(scales[c]),
                            float(biases[c]),
                        )
                        k += 1
                        store_eng.dma_start(xout[:, sl], t[:])
    nc.compile()
    return nc


def _get_scale_kernel(
    scales, biases, dtype_tag, tile_f=4096, bufs=8, engines="v", store_act=True
):
    key = (
        dtype_tag,
        tile_f,
        bufs,
        engines,
        store_act,
        tuple(np.float32(s) for s in scales),
        tuple(np.float32(b) for b in biases),
    )
    if key not in _compiled:
        _compiled[key] = _build_scale_kernel(
            scales, biases, dtype_tag, tile_f, bufs, engines, False, store_act
        )
    return _compiled[key]


LAST_RESULTS = None  # BassKernelResults of the most recent device run
PHASE_NS = None  # phase wall timings of the most recent device run


def _run_diag_affine(x, scales, biases, dtype_tag):
    global LAST_RESULTS, PHASE_NS
    import time

    from concourse.bass_utils import run_bass_kernel_spmd

    t0 = time.time()
    nc = _get_scale_kernel(scales, biases, dtype_tag)
    t1 = time.time()

    if dtype_tag == "bf16":
        import ml_dtypes

        xs = x.astype(ml_dtypes.bfloat16)
    else:
        xs = x
    in_maps = [{"x": xs[i]} for i in range(_B)]
    t2 = time.time()
    res = run_bass_kernel_spmd(nc, in_maps, core_ids=list(range(_N_CORES)))
    t3 = time.time()
    LAST_RESULTS = res
    out = np.empty((_B, _C, _H, _W), np.float32)
    for i in range(_B):
        out[i] = res.results[i]["out"]  # casts bf16 -> f32 in one pass
    t4 = time.time()
    PHASE_NS = {
        "build": int((t1 - t0) * 1e9),
        "convert": int((t2 - t1) * 1e9),
        "spmd": int((t3 - t2) * 1e9),
        "post": int((t4 - t3) * 1e9),
    }
    return out


# ---------------------------------------------------------------------------
# Entry point
# ---------------------------------------------------------------------------


def kernel(LUT=None, x=None, **kwargs):
    LUT = np.asarray(LUT, dtype=np.float32)
    x = np.ascontiguousarray(np.asarray(x, dtype=np.float32))
    assert x.shape == (_B, _C, _H, _W), x.shape

    aff = _affine_from_lut(LUT)
    if aff is not None:
        M, bias = aff
        offdiag = M - np.diag(np.diag(M))
        if np.all(offdiag == 0.0):
            scales = np.diag(M) / np.float64(_BINSIZE)
            try:
                return _run_diag_affine(
                    x, scales, bias, os.environ.get("LUT_KERNEL_DTYPE", "bf16")
                )
            except Exception:
                import traceback

                traceback.print_exc()
        # general affine (or device failure): exact on host
        inv = 1.0 / np.float64(_BINSIZE)
        xs = x * np.float32(inv)
        out = np.einsum("ck,bkhw->bchw", M.astype(np.float32), xs)
        out += bias.astype(np.float32)[None, :, None, None]
        return out.astype(np.float32, copy=False)

    return _trilinear_np(LUT, x)
